# revision 5
# baseline (speedup 1.0000x reference)
"""DoRA adapter forward kernel for 8 trn2 NeuronCores — v5 (For_i loops).

This environment is instruction-dispatch-bound (~60-100us per *static*
instruction execution; hardware-loop iterations are ~free). So the kernel
wraps all hot work in For_i loops with fixed-address tile bodies and
register-offset (ds/ts) DMA addressing, keeping the static instruction
count per rep at ~150 instead of ~6000.

Math:  dora = dora_B @ dora_A                       [OUT, IN]
       num  = weight + ALPHA * dora                 [OUT, IN]
       s    = m / sqrt(colsum_over_out(num^2))      [1, IN]
       out  = x @ (num * s)^T + bias

Sharding (4x2 grid): 4-way split of the 8192 x-rows, 2-way split of OUT.
AllReduce of the per-o-half sumsq partials (each appears MG times ->
folded 1/MG into the sqrt scale).

Per core:
  L1 (For_i ot=16): w row-tile [128,4096] f32 -> dora (PE, staged bt
      slice) -> num bf16 -> store num_bf.
  L2 (For_i it=32): DMA-transpose num_bf column-stripe into resident
      ntT_big[:, it*2048:...]; Act square+accum -> ssq[:, it] (sumsq over
      own o-half, per-partition i layout).
  s:  store ssq -> AllReduce -> sqrt/recip * m (all [128, 32] layout).
  L3 (For_i mt=16): SWDGE-cast x row-tile f32->bf16 -> store x_bf.
  L4 (For_i mt=16): inner For_i it: DMA-transpose xs stripe + scale by
      s_t[:, it]; peeled it=0 GEMM (start=True) + inner For_i it=1..31
      (staged lhsT copy + 4 accumulating matmuls, start=False); +bias,
      store out row-tile.
"""

import sys

if "/opt/trn_rl_repo" not in sys.path:
    sys.path.insert(0, "/opt/trn_rl_repo")

import numpy as np

import concourse.bass as bass
import concourse.mybir as mybir
import concourse.tile as tile
from concourse import bacc
from concourse.bass import ds, ts
from concourse.bass_utils import run_bass_kernel_spmd
from concourse.tile_rust import add_dep_helper

F32 = mybir.dt.float32
BF16 = mybir.dt.bfloat16

ALPHA = 16.0
N_CORES = 8
MG, OG = 4, 2

B_, S_, IN_FULL, OUT_FULL, R_ = 4, 2048, 4096, 4096, 16
M_FULL = B_ * S_
M_C = M_FULL // MG      # 2048 x-rows per core
O_C = OUT_FULL // OG    # 2048 out-cols per core


def build_kernel(M_C, IN, O_C, R, n_cores=N_CORES, reps=1):
    nc = bacc.Bacc("TRN2", target_bir_lowering=False, debug=False,
                   num_devices=n_cores)

    x_in = nc.dram_tensor("x_slice", [M_C, IN], F32, kind="ExternalInput")
    w_own = nc.dram_tensor("w_own", [O_C, IN], F32, kind="ExternalInput")
    bias_in = nc.dram_tensor("bias_own", [1, O_C], F32, kind="ExternalInput")
    m_in = nc.dram_tensor("m_row", [1, IN], F32, kind="ExternalInput")
    a_in = nc.dram_tensor("dora_a", [R, IN], F32, kind="ExternalInput")
    b_own = nc.dram_tensor("dora_b_own", [O_C, R], F32, kind="ExternalInput")
    out_t = nc.dram_tensor("out_slice", [M_C, O_C], F32, kind="ExternalOutput")

    num_bf = nc.dram_tensor("num_bf", [O_C, IN], BF16)
    x_bf = nc.dram_tensor("x_bf", [M_C, IN], BF16)
    s_dram = nc.dram_tensor("s_dram", [IN // 128, 128], F32)
    cc_out = nc.dram_tensor("cc_out", [IN // 128, 128], F32,
                            addr_space="Shared")

    v = dict(locals())
    with tile.TileContext(nc) as tc:
        v["tc"] = tc
        for rep in range(reps):
            if rep:
                tc.strict_bb_all_engine_barrier()
            _emit(tc, v)
    nc.compile()
    return nc


def _emit(tc, v):
    nc = v["nc"]
    IN, R = v["IN"], v["R"]
    M_C, O_C = v["M_C"], v["O_C"]
    x_in, w_own = v["x_in"], v["w_own"]
    bias_in, m_in, a_in, b_own = v["bias_in"], v["m_in"], v["a_in"], v["b_own"]
    out_t = v["out_t"]
    num_bf, x_bf = v["num_bf"], v["x_bf"]
    s_dram, cc_out = v["s_dram"], v["cc_out"]
    n_it = IN // 128
    n_ot = O_C // 128
    n_mt = M_C // 128

    with tc.tile_pool(name="const", bufs=1) as const, \
         tc.tile_pool(name="ntTp", bufs=1) as ntTp:
        ntT = ntTp.tile([128, n_it * O_C], BF16, tag="ntT")
        # ntT holds num^T as 32 column-blocks of [128, O_C] bf16
        # (128KB/partition).
        _emit_body(tc, v, const, ntT)


def _emit_body(tc, v, const, ntT):
    nc = v["nc"]
    IN, R = v["IN"], v["R"]
    M_C, O_C = v["M_C"], v["O_C"]
    x_in, w_own = v["x_in"], v["w_own"]
    bias_in, m_in, a_in, b_own = v["bias_in"], v["m_in"], v["a_in"], v["b_own"]
    out_t = v["out_t"]
    num_bf, x_bf = v["num_bf"], v["x_bf"]
    s_dram, cc_out = v["s_dram"], v["cc_out"]
    n_it = IN // 128
    n_ot = O_C // 128
    n_mt = M_C // 128

    # ---------------- setup (static) ----------------
    a_bf = const.tile([R, IN], BF16, tag="a_bf")
    nc.gpsimd.dma_start(out=a_bf[:], in_=a_in[:, :])  # SWDGE f32->bf16

    bt_f = const.tile([R, O_C], F32, tag="bt_f")
    nc.sync.dma_start(out=bt_f[:], in_=b_own.ap().rearrange("o r -> r o"))
    bt_bf = const.tile([R, O_C], BF16, tag="bt_bf")
    nc.vector.tensor_scalar_mul(bt_bf[:], bt_f[:], ALPHA)

    ssq = const.tile([128, n_it], F32, tag="ssq")
    s_t = const.tile([128, n_it], F32, tag="s_t")

    # bias replicated across partitions (ones-matmul)
    ones_row = const.tile([1, 128], F32, tag="ones_row")
    nc.gpsimd.memset(ones_row[:], 1.0)
    bias_rep = const.tile([128, O_C], F32, tag="bias_rep")
    with tc.tile_pool(name="biasp", bufs=1) as biasp, \
         tc.tile_pool(name="bias_ps", bufs=2, space="PSUM") as bias_ps:
        bias_sb = biasp.tile([1, O_C], F32, tag="bias_sb")
        nc.sync.dma_start(out=bias_sb[0:1, :], in_=bias_in[:, :])
        for oc in range(O_C // 512):
            ps_b = bias_ps.tile([128, 512], F32, tag="ps_b")
            nc.tensor.matmul(ps_b[:], lhsT=ones_row[:],
                             rhs=bias_sb[0:1, oc * 512:(oc + 1) * 512],
                             start=True, stop=True)
            nc.vector.tensor_copy(
                out=bias_rep[:, oc * 512:(oc + 1) * 512], in_=ps_b[:])

    # ---------------- L1: num = w + alpha*B@A, store num_bf -------------
    with tc.tile_pool(name="p1", bufs=1) as p1, \
         tc.tile_pool(name="p1ps", bufs=1, space="PSUM") as p1ps:
        w_t = p1.tile([128, IN], F32, tag="w_t")
        bt_st = p1.tile([R, 128], BF16, tag="bt_st")
        num_t = p1.tile([128, IN], BF16, tag="num_t")
        ps_d0 = p1ps.tile([128, IN // 2], F32, tag="ps_d0")
        ps_d1 = p1ps.tile([128, IN // 2], F32, tag="ps_d1")
        with tc.For_i(0, n_ot, 2) as i:
            for k in range(2):
                nc.sync.dma_start(out=w_t[:], in_=w_own[ts(i + k, 128), :])
                nc.vector.tensor_copy(out=bt_st[:], in_=bt_bf[:, ts(i + k, 128)])
                for q in range(IN // 512):
                    ps = ps_d0 if q < IN // 1024 else ps_d1
                    qq = q % (IN // 1024)
                    nc.tensor.matmul(
                        ps[:, qq * 512:(qq + 1) * 512],
                        lhsT=bt_st[:],
                        rhs=a_bf[:, q * 512:(q + 1) * 512],
                        start=True, stop=True)
                nc.vector.tensor_add(out=num_t[:, 0:IN // 2],
                                     in0=w_t[:, 0:IN // 2], in1=ps_d0[:])
                nc.vector.tensor_add(out=num_t[:, IN // 2:IN],
                                     in0=w_t[:, IN // 2:IN], in1=ps_d1[:])
                st_num = nc.gpsimd.dma_start(out=num_bf[ts(i + k, 128), :],
                                             in_=num_t[:])

    # ---------------- L2: ntT stripes + sumsq --------------------------
    nc.gpsimd.drain()
    tc.strict_bb_all_engine_barrier()
    with tc.tile_pool(name="p2a", bufs=1) as p2a:
        sq_scr = p2a.tile([128, O_C], BF16, tag="sq_scr")
        acc1 = p2a.tile([128, 1], F32, tag="acc1")
        with tc.For_i(0, n_it, 4) as i:
            for k in range(4):
                ld = nc.sync.dma_start_transpose(
                    ntT[:, ds((i + k) * O_C, O_C)],
                    num_bf[:, ds((i + k) * 128, 128)])
                add_dep_helper(ld.ins, st_num.ins, reason="ntT RAW on num_bf")
                nc.scalar.activation(
                    sq_scr[:], ntT[:, ds((i + k) * O_C, O_C)],
                    mybir.ActivationFunctionType.Square, 0.0, 1.0,
                    accum_out=acc1[:])
                nc.vector.tensor_copy(out=ssq[:, ds(i + k, 1)], in_=acc1[:])

    # ---------------- s = m / sqrt(MG * colsum) ------------------------
    st_s = nc.gpsimd.dma_start(
        out=s_dram.ap().rearrange("a b -> b a"), in_=ssq[:])
    cc = nc.gpsimd.collective_compute(
        "AllReduce", mybir.AluOpType.add,
        ins=[s_dram.ap()], outs=[cc_out.ap()],
        replica_groups=[list(range(N_CORES))])
    add_dep_helper(cc.ins, st_s.ins, reason="collective RAW on s_dram")
    cc_sb = const.tile([128, n_it], F32, tag="cc_sb")
    ld = nc.sync.dma_start(out=cc_sb[:], in_=cc_out.ap().rearrange("a b -> b a"))
    add_dep_helper(ld.ins, cc.ins, reason="cc_sb RAW on collective out")
    sq_s = const.tile([128, n_it], F32, tag="sq_s")
    nc.scalar.activation(sq_s[:], cc_sb[:],
                         mybir.ActivationFunctionType.Sqrt, 0.0, 1.0 / MG)
    rc_s = const.tile([128, n_it], F32, tag="rc_s")
    nc.vector.reciprocal(rc_s[:], sq_s[:])
    m_t = const.tile([128, n_it], F32, tag="m_t")
    nc.sync.dma_start(
        out=m_t[:], in_=m_in.ap().rearrange("a (c p) -> (a p) c", p=128))
    nc.vector.tensor_mul(out=s_t[:], in0=rc_s[:], in1=m_t[:])

    # ------- L4: x cast + xs transpose + GEMM, fused per m-tile --------
    with tc.tile_pool(name="p4", bufs=1) as p4, \
         tc.tile_pool(name="p4ps", bufs=1, space="PSUM") as p4ps:
        xb = p4.tile([128, IN], BF16, tag="xb")
        xs_big = p4.tile([128, n_it * 128], BF16, tag="xs_big")
        lh_st = p4.tile([128, 128], BF16, tag="lh_st")
        o_sb = p4.tile([128, O_C], F32, tag="o_sb")
        ps_o = p4ps.tile([128, O_C], F32, tag="ps_o")

        def xs_make(i0, dyn):
            """DMA-transpose + scale stripe i0 (offset expr) into xs_big."""
            off = (lambda e: ds(e * 128, 128)) if dyn else \
                  (lambda e: slice(e * 128, (e + 1) * 128))
            ld = nc.sync.dma_start_transpose(
                xs_big[:, off(i0)], x_bf[ts(mi, 128), off(i0)])
            add_dep_helper(ld.ins, st_x.ins, reason="xs RAW on x_bf")
            nc.vector.tensor_scalar_mul(
                xs_big[:, off(i0)], xs_big[:, off(i0)],
                s_t[:, ds(i0, 1) if dyn else slice(i0, i0 + 1)])

        def gemm_step(i0, dyn, start, stop):
            off = (lambda e: ds(e * 128, 128)) if dyn else \
                  (lambda e: slice(e * 128, (e + 1) * 128))
            nc.vector.tensor_copy(out=lh_st[:], in_=xs_big[:, off(i0)])
            for q in range(O_C // 512):
                rhs = (ntT[:, ds(i0 * O_C + q * 512, 512)] if dyn else
                       ntT[:, i0 * O_C + q * 512:i0 * O_C + (q + 1) * 512])
                nc.tensor.matmul(
                    ps_o[:, q * 512:(q + 1) * 512],
                    lhsT=lh_st[:], rhs=rhs,
                    start=start, stop=stop, skip_group_check=True)

        with tc.For_i(0, n_mt) as mi:
            # x f32 -> bf16 for this m-tile (SWDGE), then transposed reads
            nc.gpsimd.dma_start(out=xb[:], in_=x_in[ts(mi, 128), :])
            st_x = nc.gpsimd.dma_start(out=x_bf[ts(mi, 128), :], in_=xb[:])
            # peeled it=0: transpose+scale, start accumulation
            xs_make(0, False)
            gemm_step(0, False, True, False)
            with tc.For_i(1, n_it - 1, 6) as i:
                for k in range(6):
                    xs_make(i + k, True)
                    gemm_step(i + k, True, False, False)
            xs_make(n_it - 1, False)
            gemm_step(n_it - 1, False, False, True)
            nc.vector.tensor_add(out=o_sb[:], in0=ps_o[:], in1=bias_rep[:])
            nc.gpsimd.dma_start(out=out_t[ts(mi, 128), :], in_=o_sb[:])
    nc.gpsimd.drain()


_NC_CACHE = {}


def get_nc(M_C=M_C, IN=IN_FULL, O_C=O_C, R=R_):
    key = (M_C, IN, O_C, R)
    if key not in _NC_CACHE:
        _NC_CACHE[key] = build_kernel(M_C, IN, O_C, R)
    return _NC_CACHE[key]


def make_in_maps(x, weight, bias, m, dora_A, dora_B):
    x = np.ascontiguousarray(np.asarray(x, dtype=np.float32))
    weight = np.ascontiguousarray(np.asarray(weight, dtype=np.float32))
    bias = np.ascontiguousarray(np.asarray(bias, dtype=np.float32))
    m = np.ascontiguousarray(np.asarray(m, dtype=np.float32))
    dora_A = np.ascontiguousarray(np.asarray(dora_A, dtype=np.float32))
    dora_B = np.ascontiguousarray(np.asarray(dora_B, dtype=np.float32))
    xf = x.reshape(M_FULL, IN_FULL)
    in_maps = []
    for c in range(N_CORES):
        g, h = divmod(c, OG)
        o0 = h * O_C
        im = {
            "x_slice": np.ascontiguousarray(xf[g * M_C:(g + 1) * M_C]),
            "w_own": np.ascontiguousarray(weight[o0:o0 + O_C]),
            "bias_own": np.ascontiguousarray(bias[o0:o0 + O_C].reshape(1, O_C)),
            "m_row": np.ascontiguousarray(m.reshape(1, IN_FULL)),
            "dora_a": dora_A,
            "dora_b_own": np.ascontiguousarray(dora_B[o0:o0 + O_C]),
        }
        in_maps.append(im)
    return in_maps


def kernel(x, weight, bias, m, dora_A, dora_B, _trace=False, _trace_kwargs=None):
    in_maps = make_in_maps(x, weight, bias, m, dora_A, dora_B)
    res = run_bass_kernel_spmd(
        get_nc(), in_maps, core_ids=list(range(N_CORES)),
        trace=_trace, **(_trace_kwargs or {}))
    out = np.empty((M_FULL, OUT_FULL), np.float32)
    for c in range(N_CORES):
        g, h = divmod(c, OG)
        out[g * M_C:(g + 1) * M_C, h * O_C:(h + 1) * O_C] = \
            res.results[c]["out_slice"]
    ret = out.reshape(B_, S_, OUT_FULL)
    if _trace:
        return ret, res
    return ret
